# revision 16
# baseline (speedup 1.0000x reference)
"""DiffuseRouter kernel for 8 TRN2 NeuronCores — PE + grouped-plane layout.

out[b] = (1/3) * sum of 28 expert planes; data-parallel over B=8.

Host side re-lays each core's 28 planes as x2[p, e, 2560] (partition-major)
so one SWDGE casting DMA covers a GROUP of planes with per-partition
contiguous runs of group_size*10240 B — 5x fewer, 5x bigger descriptors
than per-plane loads.  Group sizes [5,5,5,5,5,2,1] keep the final groups
small so the last matmuls/copies/stores start right behind the stream.
Accumulation: TensorEngine identity-matmuls (bf16 in, fp32 PSUM), one PSUM
bank + one SBUF result tile per 512-col chunk so the tail chains pipeline.
"""

import numpy as np
import ml_dtypes

import concourse.bacc as bacc
import concourse.tile as tile
from concourse import mybir
from concourse.bass import MemorySpace
from concourse.bass_utils import run_bass_kernel_spmd

N_CORES = 8
E_TOTAL = 28
L, D = 256, 1280
P = 128
FD = (L // P) * D  # 2560 per plane per partition
NCH = 5
CH = FD // NCH  # 512
SCALE = 1.0 / 3.0
GROUPS = [5, 5, 5, 5, 5, 2, 1]
assert sum(GROUPS) == E_TOTAL

_NC_CACHE = None


def _build_nc():
    nc = bacc.Bacc(
        "TRN2", target_bir_lowering=False, debug=False, enable_partition_id=False
    )
    x = nc.dram_tensor(
        "x", [P, E_TOTAL, FD], mybir.dt.float32, kind="ExternalInput"
    )
    ident = nc.dram_tensor("ident", [P, P], mybir.dt.bfloat16, kind="ExternalInput")
    out = nc.dram_tensor("out", [L, D], mybir.dt.float32, kind="ExternalOutput")

    x_t = x.ap().rearrange("p e f -> p (e f)")
    out_t = out.ap().rearrange("(p a) d -> p (a d)", a=2)

    with tile.TileContext(nc) as tc:
        with (
            tc.tile_pool(name="in", bufs=3) as pin,
            tc.tile_pool(name="w", bufs=1) as pw,
            tc.tile_pool(name="res", bufs=1) as pres,
            tc.tile_pool(name="ps", bufs=1, space=MemorySpace.PSUM) as pps,
        ):
            idt = pw.tile([P, P], mybir.dt.bfloat16, name="idt", tag="idt")
            nc.sync.dma_start(out=idt[:], in_=ident.ap())
            psums = [
                pps.tile([P, CH], mybir.dt.float32, name=f"ps{c}", tag=f"ps{c}")
                for c in range(NCH)
            ]
            ress = [
                pres.tile([P, CH], mybir.dt.float32, name=f"res{c}", tag=f"res{c}")
                for c in range(NCH)
            ]

            e0 = 0
            for gi, gs in enumerate(GROUPS):
                t = pin.tile([P, gs * FD], mybir.dt.bfloat16)
                nc.gpsimd.dma_start(
                    out=t[:], in_=x_t[:, e0 * FD : (e0 + gs) * FD]
                )
                for j in range(gs):
                    e = e0 + j
                    for c in range(NCH):
                        nc.tensor.matmul(
                            psums[c][:],
                            idt[:],
                            t[:, j * FD + c * CH : j * FD + (c + 1) * CH],
                            start=(e == 0),
                            stop=(e == E_TOTAL - 1),
                        )
                        if e == E_TOTAL - 1:
                            sl = slice(c * CH, (c + 1) * CH)
                            nc.vector.tensor_scalar_mul(
                                ress[c][:], psums[c][:], SCALE
                            )
                            eng = nc.sync if c % 2 == 0 else nc.scalar
                            eng.dma_start(out=out_t[:, sl], in_=ress[c][:])
                e0 += gs
    nc.compile()
    return nc


def _get_nc():
    global _NC_CACHE
    if _NC_CACHE is None:
        _NC_CACHE = _build_nc()
    return _NC_CACHE


def _prep_core(e0, e1, e2, b):
    # [28, 256, 1280] -> [128, 28, 2560]: partition p holds rows 2p,2p+1 of
    # every plane, planes contiguous per partition (grouped-descriptor DMA).
    xb = np.concatenate([e0[:, b], e1[:, b], e2[:, b]], axis=0)
    xr = xb.reshape(E_TOTAL, P, FD).transpose(1, 0, 2)
    return np.ascontiguousarray(xr)


def _run(inputs, trace=False, trace_kwargs=None):
    e0 = np.asarray(inputs["expert_emb_0"], dtype=np.float32)
    e1 = np.asarray(inputs["expert_emb_1"], dtype=np.float32)
    e2 = np.asarray(inputs["expert_emb_2"], dtype=np.float32)
    B = e0.shape[1]
    assert B == N_CORES, f"expected B == {N_CORES}, got {B}"

    ident = np.eye(P, dtype=ml_dtypes.bfloat16)
    in_maps = [
        {"x": _prep_core(e0, e1, e2, b), "ident": ident} for b in range(B)
    ]

    kw = {}
    if trace:
        kw["trace"] = True
        if trace_kwargs:
            kw.update(trace_kwargs)
    try:
        res = run_bass_kernel_spmd(_get_nc(), in_maps, list(range(N_CORES)), **kw)
    except Exception:
        res = run_bass_kernel_spmd(_get_nc(), in_maps, list(range(N_CORES)), **kw)
    out = np.stack([res.results[b]["out"] for b in range(B)], axis=0)
    return out.astype(np.float32, copy=False), res


def kernel(**inputs) -> np.ndarray:
    out, _ = _run(inputs, trace=False)
    return out
